# revision 19
# baseline (speedup 1.0000x reference)
"""Trainium2 Bass kernel: quantized BasicBlock (quant-conv3x3 -> bn -> relu ->
quant-conv3x3 -> bn -> +residual -> relu).

Sharding: data-parallel over the batch dim of x across 8 NeuronCores (8 images
per core).  Weight quantization (centroid/deviation pipeline) is replicated on
every core, computed on-device.

Math notes:
  - jnp.round (round-half-even) is implemented with the fp32 magic-number
    trick: rne(v) = (v + 1.5*2^23) - 1.5*2^23 for |v| < 2^22.
  - rne(clip(x, -63.5, 63.5)) == clip(rne(x), -64, 64) pointwise, which fuses
    the deviation clamp into two tensor_scalar ops.
  - Quantized weights are integer "levels" dev+cent = k/8 with |k| < 2048,
    exactly representable in fp16.  The global scale `step` is folded into the
    BN scale vector, so matmuls run in fp16 (2x faster than fp32 on the PE)
    with fp32 PSUM accumulation and no weight-precision loss.

Schedule notes (from perfetto traces):
  - The PE queue is strictly in-order; anything emitted ahead of ready work
    stalls it.  BN row DMAs ride the scalar/gpsimd queues so the 16 K=1
    redistribution matmuls (fp16: fp32 PE matmuls trigger a ~7us half-clock
    throttle window) retire in the first ~3us.
  - w1 goes first on the sync ring in 4 descriptors; absmax reduces are
    per-descriptor so istep is ready ~2us after w1 lands.
  - Junk warmup matmuls keep the PE clock pinned at full rate through the
    head's DMA/quant-chain wait.
  - conv matmuls are r-paired (each weight feeds both row chunks) and the
    image-0 m0 conv starts on the k0 tap group while the k1 chain runs.
"""

import sys

for _p in ("/opt/trn_rl_repo",):
    if _p not in sys.path:
        sys.path.insert(0, _p)

from contextlib import ExitStack

import numpy as np

import concourse.tile as tile
from concourse import bacc, mybir
from concourse.bass_utils import run_bass_kernel_spmd
from concourse.masks import make_identity

P = 128
B, C, H, W = 64, 256, 28, 28
NCORES = 8
BPC = B // NCORES          # images per core
CK = C // P                # channel chunks (2)
TAPS = 9
HP, WP = H + 2, W + 2      # zero-padded spatial 30x30
NR = H // 2                # rows per psum chunk (14)
NN = NR * W                # matmul free dim (392)
F32 = mybir.dt.float32
F16 = mybir.dt.float16

MAGIC = 12582912.0         # 1.5 * 2**23  (fp32 RNE round-to-int trick)
HALF_LVLS = 127.0
LV = 8.0                   # 2**(NUM_BITS-1)
CSTEP = HALF_LVLS / LV     # 15.875
DEVW = 64.0                # post-rne clip bound (rne-clip-63.5 fused)
BN_EPS = 1e-5

AF = mybir.ActivationFunctionType
OP = mybir.AluOpType
AX = mybir.AxisListType


def _emit(nc, tc, ctx, td):
    """Emit the whole per-core program.  td: dict of DRAM tensor handles."""
    const = ctx.enter_context(tc.tile_pool(name="const", bufs=1))
    bnp = ctx.enter_context(tc.tile_pool(name="bnp", bufs=2))
    wbig = ctx.enter_context(tc.tile_pool(name="wbig", bufs=1))
    whalf = ctx.enter_context(tc.tile_pool(name="whalf", bufs=4))
    wqp = ctx.enter_context(tc.tile_pool(name="wqp", bufs=1))
    wtp = ctx.enter_context(tc.tile_pool(name="wtp", bufs=1))
    tpq = ctx.enter_context(tc.tile_pool(name="tpq", bufs=1, space="PSUM"))
    tpp = ctx.enter_context(tc.tile_pool(name="tpp", bufs=1, space="PSUM"))
    psp = ctx.enter_context(tc.tile_pool(name="psp", bufs=6, space="PSUM"))
    pxf = ctx.enter_context(tc.tile_pool(name="pxf", bufs=6))
    pxp = ctx.enter_context(tc.tile_pool(name="pxp", bufs=5))
    phh = ctx.enter_context(tc.tile_pool(name="phh", bufs=5))
    pyy = ctx.enter_context(tc.tile_pool(name="pyy", bufs=3))
    pep = ctx.enter_context(tc.tile_pool(name="pep", bufs=3))

    ident16 = const.tile([P, P], F16, name="ident16", tag="ident16")
    make_identity(nc, ident16)
    ident32 = const.tile([P, P], F32, name="ident32", tag="ident32")
    make_identity(nc, ident32)
    ones32 = const.tile([1, P], F32, name="ones32", tag="ones32")
    nc.gpsimd.memset(ones32[:], 1.0)
    magicv = const.tile([P, 1], F32, name="magicv", tag="magicv")
    nc.gpsimd.memset(magicv[:], MAGIC)
    nmagicv = const.tile([P, 1], F32, name="nmagicv", tag="nmagicv")
    nc.gpsimd.memset(nmagicv[:], -MAGIC)
    # warm the ScalarE activation tables during the initial DMA wait so the
    # one-time ACT_TABLE_LOADs don't block the quant chain later
    scr = const.tile([P, 1], F32, name="scr", tag="scr")
    nc.scalar.activation(scr[:], magicv[:], AF.Sqrt)
    nc.scalar.activation(scr[:], magicv[:], AF.Relu)
    ones16 = const.tile([1, P], F16, name="ones16", tag="ones16")
    nc.gpsimd.memset(ones16[:], 1.0)
    # warm the DVE with dependency-free ops: several op flavors pay a
    # multi-us first-use cost (observed: 4.9-17us for the first big CAST /
    # tensor_scalar); run one of each during the initial DMA wait instead
    dvew = const.tile([P, 2], F32, name="dvew", tag="dvew")
    nc.vector.memset(dvew[:], 0.0)
    dvew16 = const.tile([P, 2], F16, name="dvew16", tag="dvew16")
    nc.vector.tensor_copy(dvew16[:], dvew[:])
    dw2 = const.tile([P, 2], F32, name="dw2", tag="dw2")
    nc.vector.tensor_scalar(dw2[:], dvew[:], 1.0, 0.5, OP.mult, OP.add)
    nc.vector.tensor_scalar(dw2[:], dvew[:], 1.0, 0.5, OP.subtract, OP.min)
    nc.vector.tensor_scalar_add(dw2[:], dvew[:], 0.5)
    nc.vector.tensor_scalar_sub(dw2[:], dvew[:], 0.5)
    nc.vector.tensor_scalar_mul(dw2[:], dvew[:], 0.5)
    nc.vector.tensor_scalar(dw2[:], dvew[:], MAGIC, CSTEP, OP.subtract, OP.mult)
    nc.vector.tensor_add(dw2[:], dvew[:], dvew[:])
    nc.vector.tensor_sub(dw2[:], dvew[:], dvew[:])
    nc.vector.tensor_mul(dw2[:], dvew[:], dvew[:])
    nc.vector.tensor_max(dw2[:], dvew[:], dvew[:])
    nc.vector.scalar_tensor_tensor(dw2[:], dvew[:], -1.0, dvew[:], OP.max, OP.add)
    nc.vector.reciprocal(dw2[:], dvew[:])
    dw1 = const.tile([P, 1], F32, name="dw1", tag="dw1")
    nc.vector.tensor_reduce(dw1[:], dvew[:], axis=AX.XY, op=OP.max,
                            apply_absolute_value=True)
    nc.vector.tensor_reduce(dw1[:], dvew[:], axis=AX.X, op=OP.add)

    wT = {}      # wT[j][k] : [P(ci), CK(m), TAPS, P(co)] fp16
    inv_s = {}   # BN scale with quant step folded in: [P, CK]
    bvec = {}    # BN bias: [P, CK]
    _w32 = {}
    _wq = {}
    _istep = {}
    _step = {}
    _inv = {}
    _bnrows = {}
    _psB = {}

    # ---------------- image loads ------------------------------------------
    x_view = td["x"].ap().rearrange("b (c p) h w -> b p c h w", p=P)
    y_view = td["y"].ap().rearrange("b (c p) h w -> b p c h w", p=P)
    xf_t = [None] * BPC
    xp_t = [None] * BPC
    h_t = [None] * BPC

    def load_x_dma(i):
        xf = pxf.tile([P, CK, H, W], F32, name=f"xf{i}", tag="xf")
        nc.sync.dma_start(xf[:], x_view[i])
        xf_t[i] = xf

    def pad_x(i):
        xp = pxp.tile([P, CK, HP, WP], F16, name=f"xp{i}", tag="xp")
        nc.gpsimd.memset(xp[:], 0.0)
        nc.scalar.copy(xp[:, :, 1 : 1 + H, 1 : 1 + W], xf_t[i][:])
        xp_t[i] = xp

    def load_x(i):
        load_x_dma(i)
        pad_x(i)

    # ---------------- per-weight quantization ------------------------------
    def quant_dma(j):
        """Weight DMA: 4 descriptors (per co-chunk x ci-half) so the absmax
        reduces pipeline with the transfer."""
        w32 = wbig.tile([P, CK, C, TAPS], F32, name=f"w32_{j}", tag="wbig")
        wsrc = td[f"w{j}"].ap().rearrange("(c p) ci kh kw -> p c ci (kh kw)", p=P)
        for c in range(CK):
            for k in range(CK):
                ks = slice(k * P, (k + 1) * P)
                nc.sync.dma_start(w32[:, c, ks, :], wsrc[:, c, ks, :])
        _w32[j] = w32

    def quant_absmax(j):
        """Global absmax -> step/istep (reduces on DVE, combine on PE)."""
        w32 = _w32[j]
        pmq = []
        for c in range(CK):
            for k in range(CK):
                ks = slice(k * P, (k + 1) * P)
                ph = bnp.tile([P, 1], F32, name=f"pmq{j}_{c}_{k}", tag="pmq")
                nc.vector.tensor_reduce(
                    ph[:], w32[:, c, ks, :], axis=AX.XY, op=OP.max,
                    apply_absolute_value=True,
                )
                pmq.append(ph)
        pa = bnp.tile([P, 1], F32, name=f"pa{j}", tag="pa")
        nc.vector.tensor_max(pa[:], pmq[0][:], pmq[1][:])
        pb = bnp.tile([P, 1], F32, name=f"pb{j}", tag="pb")
        nc.vector.tensor_max(pb[:], pmq[2][:], pmq[3][:])
        pm = bnp.tile([P, 1], F32, name=f"pm{j}", tag="pm")
        nc.vector.tensor_max(pm[:], pa[:], pb[:])
        # cross-partition max via PE: transpose [128,1]->[1,128], reduce,
        # then broadcast back with a K=1 ones matmul.  Short fp32 burst only
        # (sustained fp32 PE work triggers a multi-us half-clock window).
        pmt = tpq.tile([1, P], F32, name=f"pmt{j}", tag="tp32")
        nc.tensor.transpose(pmt[:], pm[:], ident32[:])
        sm = bnp.tile([1, 1], F32, name=f"sm{j}", tag="sm")
        nc.vector.tensor_reduce(sm[:], pmt[:], axis=AX.X, op=OP.max)
        pmb = tpq.tile([P, 1], F32, name=f"pmb{j}", tag="tp32")
        nc.tensor.matmul(pmb[:], ones32[:], sm[:])
        pmax = bnp.tile([P, 1], F32, name=f"pmax{j}", tag="pmax")
        nc.vector.tensor_copy(pmax[:], pmb[:])
        step = const.tile([P, 1], F32, name=f"step{j}", tag=f"step{j}")
        nc.vector.tensor_scalar_mul(step[:], pmax[:], 1.0 / HALF_LVLS)
        _step[j] = step
        rmax = bnp.tile([P, 1], F32, name=f"rmax{j}", tag="rmax")
        nc.vector.reciprocal(rmax[:], pmax[:])
        istep = const.tile([P, 1], F32, name=f"istep{j}", tag=f"istep{j}")
        nc.vector.tensor_scalar_mul(istep[:], rmax[:], HALF_LVLS)
        _istep[j] = istep
        if j == 1:
            i16 = const.tile([P, 1], F16, name="istep16", tag="istep16")
            nc.vector.tensor_copy(i16[:], istep[:])
            _istep16[0] = i16

        wq = wqp.tile([P, CK, C, TAPS], F16, name=f"wq{j}", tag=f"wq{j}")
        _wq[j] = wq
        wT[j] = []
        for k in range(CK):
            wt = wtp.tile([P, CK, TAPS, P], F16, name=f"wT{j}_{k}", tag=f"wT{j}_{k}")
            wT[j].append(wt)

    def fold_ivs(j):
        """inv_s = inv * step (after both bn_math(j) and quant_absmax(j))."""
        ivs = const.tile([P, CK], F32, name=f"ivs{j}", tag=f"ivs{j}")
        nc.vector.tensor_scalar_mul(ivs[:], _inv[j][:], _step[j][:, 0:1])
        inv_s[j] = ivs

    def bn_prep(j):
        """BN row DMAs (sync ring, behind w1) + fp16 casts on DVE."""
        rows = {}
        for v in ("gamma", "beta", "mean", "var"):
            rv = bnp.tile([1, C], F32, name=f"{v}{j}", tag=f"{v}{j}")
            nc.sync.dma_start(rv[:], td[f"{v}{j}"].ap().unsqueeze(0))
            r16 = bnp.tile([1, C], F16, name=f"{v}16_{j}", tag=f"{v}16_{j}")
            nc.scalar.copy(r16[:], rv[:])
            rows[v] = r16
        _bnrows[j] = rows

    def bn_mms(j):
        """K=1 fp16 redistribution matmuls: [1,256] rows -> [128, 2]."""
        rows = _bnrows[j]
        psB = tpq.tile([P, 4 * CK], F32, name=f"psB{j}", tag="tp32")
        for v, key in enumerate(("gamma", "beta", "mean", "var")):
            row = rows[key]
            for c in range(CK):
                nc.tensor.matmul(
                    psB[:, v * CK + c : v * CK + c + 1],
                    row[0:1, c * P : (c + 1) * P],
                    ones16[0:1, 0:1],
                )
        _psB[j] = psB

    def bn_math(j):
        psB = _psB[j]
        bn4 = bnp.tile([P, 4, CK], F32, name=f"bn4_{j}", tag=f"bn4_{j}")
        nc.vector.tensor_copy(bn4[:], psB[:].rearrange("p (v c) -> p v c", c=CK))
        gvp, bevp, muvp, vavp = (bn4[:, v, :] for v in range(4))
        tv = bnp.tile([P, CK], F32, name=f"tv{j}", tag="btmp")
        nc.vector.tensor_scalar_add(tv[:], vavp, BN_EPS)
        rv = bnp.tile([P, CK], F32, name=f"rv{j}", tag="btmp")
        nc.vector.reciprocal(rv[:], tv[:])
        sv = bnp.tile([P, CK], F32, name=f"sv{j}", tag="btmp")
        nc.scalar.activation(sv[:], rv[:], AF.Sqrt)           # rsqrt(var+eps)
        inv = const.tile([P, CK], F32, name=f"inv{j}", tag=f"inv{j}")
        nc.vector.tensor_mul(inv[:], sv[:], gvp)              # gamma * rsqrt
        mi = bnp.tile([P, CK], F32, name=f"mi{j}", tag="btmp")
        nc.vector.tensor_mul(mi[:], muvp, inv[:])
        bv = const.tile([P, CK], F32, name=f"bv{j}", tag=f"bv{j}")
        nc.vector.tensor_sub(bv[:], bevp, mi[:])              # beta - mean*inv
        bvec[j] = bv
        _inv[j] = inv

    def chain_scalar(j, c, k):
        """Scalar-engine part of the quant chain for chunk (c, k):
        wl3 = rne(w * istep) - MAGIC.  The +-127 clip is redundant:
        |w*istep| <= 127*(1+2^-23) by construction, and rne of that is 127."""
        w32, istep = _w32[j], _istep[j]
        ks = slice(k * P, (k + 1) * P)
        wlr = whalf.tile([P, P, TAPS], F32, name=f"wlr{j}_{c}_{k}", tag="wh")
        nc.scalar.activation(
            wlr[:], w32[:, c, ks, :], AF.Identity,
            bias=magicv[:, 0:1], scale=istep[:, 0:1],
        )
        return wlr

    def chain_vec(j, c, k, wlr):
        """DVE part: wl3 -> grain means -> centroid -> fused clip/rne -> wq.
        (The centroid's +-8 clip is also redundant: |gm|/9/cstep <= 8.)"""
        wq = _wq[j]
        ks = slice(k * P, (k + 1) * P)
        wl3 = whalf.tile([P, P, TAPS], F32, name=f"wl3{j}_{c}_{k}", tag="wh")
        nc.vector.tensor_scalar_sub(wl3[:], wlr[:], MAGIC)
        gm = bnp.tile([P, P], F32, name=f"gm{j}_{c}_{k}", tag="gm")
        nc.vector.tensor_reduce(gm[:], wl3[:], axis=AX.X, op=OP.add)
        c1 = bnp.tile([P, P], F32, name=f"c1{j}_{c}_{k}", tag="c1")
        nc.vector.tensor_scalar(
            c1[:], gm[:], 1.0 / (TAPS * CSTEP), MAGIC, OP.mult, OP.add
        )
        cent = bnp.tile([P, P], F32, name=f"cent{j}_{c}_{k}", tag="cent")
        nc.vector.tensor_scalar(cent[:], c1[:], MAGIC, CSTEP, OP.subtract, OP.mult)
        mc = bnp.tile([P, P], F32, name=f"mc{j}_{c}_{k}", tag="mc")
        nc.vector.tensor_scalar(mc[:], cent[:], -1.0, MAGIC, OP.mult, OP.add)
        centb = cent.unsqueeze(2).broadcast_to((P, P, TAPS))
        mcb = mc.unsqueeze(2).broadcast_to((P, P, TAPS))
        t1 = whalf.tile([P, P, TAPS], F32, name=f"t1{j}_{c}_{k}", tag="wh")
        nc.vector.tensor_add(t1[:], wl3[:], mcb)          # wl3 + MAGIC - cent
        t2 = whalf.tile([P, P, TAPS], F32, name=f"t2{j}_{c}_{k}", tag="wh")
        nc.vector.tensor_scalar(t2[:], t1[:], MAGIC, DEVW, OP.subtract, OP.min)
        # wq = max(t2, -64) + cent   (fp16-exact: |8*wq| <= 1528 < 2048)
        nc.vector.scalar_tensor_tensor(
            wq[:, c, ks, :], t2[:], -DEVW, centb, OP.max, OP.add
        )

    def chain_transpose(j, c, k):
        """PE-transpose the 9 taps of chunk (m=c, k): [co,ci] -> [ci,co]."""
        wq = _wq[j]
        for t0 in (0, 4, 8):
            nb = min(4, TAPS - t0)
            pst = tpp.tile([P, nb, P], F16, name=f"pst{j}_{c}_{k}_{t0}", tag="tp")
            for dt in range(nb):
                nc.tensor.transpose(
                    pst[:, dt, :],
                    wq[:, c, k * P : (k + 1) * P, t0 + dt],
                    ident16[:],
                )
            nc.scalar.copy(wT[j][k][:, c, t0 : t0 + nb, :], pst[:])

    # ---------------- convolutions -----------------------------------------
    def conv_mms(ps_r, src16, wTj, m, ks=None):
        """Accumulate the 3x3xC conv into the psum chunks in ps_r (keyed by
        r).  Contiguous 9-tap blocks per bank (interleaving banks MM-by-MM
        costs ~35ns/MM); k-major so a k-half can start before the other
        half's weights exist."""
        klist = list(range(CK)) if ks is None else list(ks)
        for k in klist:
            for r in ps_r:
                r0 = r * NR
                for t in range(TAPS):
                    dh, dw = t // 3, t % 3
                    nc.tensor.matmul(
                        ps_r[r][:],
                        wTj[k][:, m, t, :],
                        src16[:, k, r0 + dh : r0 + dh + NR, dw : dw + W],
                        start=(k == 0 and t == 0),
                        stop=(k == CK - 1 and t == TAPS - 1),
                    )

    def conv1(i, ms=None, ks=None, ps_store=None):
        if ms is None or ms == [0]:
            hh = phh.tile([P, CK, HP, WP], F16, name=f"h{i}", tag="h")
            nc.gpsimd.memset(hh[:], 0.0)
            h_t[i] = hh
        hh = h_t[i]
        for m in (ms if ms is not None else range(CK)):
            if ks is None or 0 in ks:
                ps_r = {r: psp.tile([P, NN], F32, name=f"ps1_{i}_{m}_{r}", tag="ps")
                        for r in range(2)}
                if ps_store is not None:
                    ps_store[i, m] = ps_r
            else:
                ps_r = ps_store[i, m]
            conv_mms(ps_r, xp_t[i], wT[1], m, ks=ks)
            if ks is not None and CK - 1 not in ks:
                continue
            for r in range(2):
                r0 = r * NR
                nc.scalar.activation(
                    hh[:, m, 1 + r0 : 1 + r0 + NR, 1 : 1 + W],
                    ps_r[r].rearrange("p (r w) -> p r w", w=W),
                    AF.Relu,
                    bias=bvec[1][:, m : m + 1],
                    scale=inv_s[1][:, m : m + 1],
                )

    def conv2_fine(i, m):
        """Last chunk: 4 quarter-height psums so the final epilogue chain is
        short and the tail drains fast."""
        yf = pyy.tile([P, H, W], F32, name=f"y{i}_{m}", tag="y")
        NQ = NR // 2                  # 7 rows
        ps_q = {q: psp.tile([P, NQ * W], F32, name=f"psq_{i}_{m}_{q}", tag="ps")
                for q in range(4)}
        for k in range(CK):
            for q in range(4):
                q0 = q * NQ
                for t in range(TAPS):
                    dh, dw = t // 3, t % 3
                    nc.tensor.matmul(
                        ps_q[q][:],
                        wT[2][k][:, m, t, :],
                        h_t[i][:, k, q0 + dh : q0 + dh + NQ, dw : dw + W],
                        start=(k == 0 and t == 0),
                        stop=(k == CK - 1 and t == TAPS - 1),
                    )
        for q in range(4):
            q0 = q * NQ
            t2 = pep.tile([P, NQ * W], F32, name=f"t2q_{i}_{m}_{q}", tag="t2")
            nc.scalar.activation(
                t2[:], ps_q[q][:], AF.Identity,
                bias=bvec[2][:, m : m + 1], scale=inv_s[2][:, m : m + 1],
            )
            u = pep.tile([P, NQ * W], F32, name=f"uq_{i}_{m}_{q}", tag="u")
            xflat = xf_t[i][:, m, q0 : q0 + NQ, :].rearrange("p r w -> p (r w)")
            nc.vector.tensor_add(u[:], t2[:], xflat)
            nc.scalar.activation(
                yf[:, q0 : q0 + NQ, :], u.rearrange("p (r w) -> p r w", w=W),
                AF.Relu,
            )
            nc.sync.dma_start(y_view[i][:, m, q0 : q0 + NQ, :], yf[:, q0 : q0 + NQ, :])

    def conv2(i):
        for m in range(CK):
            if i == BPC - 1 and m == CK - 1:
                conv2_fine(i, m)
                continue
            yf = pyy.tile([P, H, W], F32, name=f"y{i}_{m}", tag="y")
            ps_r = {r: psp.tile([P, NN], F32, name=f"ps2_{i}_{m}_{r}", tag="ps")
                    for r in range(2)}
            conv_mms(ps_r, h_t[i], wT[2], m)
            for r in range(2):
                r0 = r * NR
                t2 = pep.tile([P, NN], F32, name=f"t2_{i}_{m}_{r}", tag="t2")
                nc.scalar.activation(
                    t2[:],
                    ps_r[r][:],
                    AF.Identity,
                    bias=bvec[2][:, m : m + 1],
                    scale=inv_s[2][:, m : m + 1],
                )
                u = pep.tile([P, NN], F32, name=f"u_{i}_{m}_{r}", tag="u")
                xflat = xf_t[i][:, m, r0 : r0 + NR, :].rearrange("p r w -> p (r w)")
                nc.vector.tensor_add(u[:], t2[:], xflat)
                nc.scalar.activation(
                    yf[:, r0 : r0 + NR, :],
                    u.rearrange("p (r w) -> p r w", w=W),
                    AF.Relu,
                )
                if i >= BPC - 2:
                    nc.sync.dma_start(
                        y_view[i][:, m, r0 : r0 + NR, :], yf[:, r0 : r0 + NR, :]
                    )
            if i < BPC - 2:
                nc.sync.dma_start(y_view[i][:, m], yf[:])

    def pe_warmup(n, gated=False):
        """Junk matmuls to hold the PE clock at full rate through the head's
        DMA wait, so real matmuls start warm.  gated=True makes them depend
        on istep so the scheduler cannot run them before the absmax path."""
        for i in range(n):
            scr_ps = psp.tile([P, NN], F32, name=f"warm{_wuid[0]}", tag="ps")
            _wuid[0] += 1
            if gated:
                rhsb = _istep16[0].broadcast_to((P, 3 * P))
            else:
                rhsb = ident16.unsqueeze(1).broadcast_to((P, 3, P))
            nc.tensor.matmul(scr_ps[:, 0 : 3 * P], ident16[:], rhsb)

    _wuid = [0]
    _istep16 = [None]

    # ---------------- emission order (engine priority) ---------------------
    # sync ring: w1 (4 descs) -> x0 -> x1 -> w2 -> x2..x7
    # scalar queue: bn1 rows; gpsimd queue: bn2 rows.
    quant_dma(1)
    bn_prep(1)
    bn_prep(2)
    load_x_dma(0)
    load_x_dma(1)
    quant_dma(2)
    for i in range(2, BPC):
        load_x_dma(i)

    pe_warmup(46)
    quant_absmax(1)          # DVE: reduces..istep + istep16; PE: T + bcast
    bn_mms(1)                # PE ~20.5us; rows land ~18.4 behind w1
    bn_mms(2)
    pe_warmup(64, gated=True)
    # chain c0: scalar ops for both k first, then DVE chains; transpose k0
    # early so image-0 m0 k0 taps start while the k1 chain runs on DVE.
    wl3_00 = chain_scalar(1, 0, 0)
    wl3_01 = chain_scalar(1, 0, 1)
    pad_x(0)
    pad_x(1)
    chain_vec(1, 0, 0, wl3_00)
    chain_transpose(1, 0, 0)
    ps_store = {}
    conv1(0, ms=[0], ks=[0], ps_store=ps_store)
    bn_math(1)               # DVE after chain k0; needs psB from bn_mms
    fold_ivs(1)
    pe_warmup(12, gated=True)
    chain_vec(1, 0, 1, wl3_01)
    chain_transpose(1, 0, 1)
    conv1(0, ms=[0], ks=[1], ps_store=ps_store)
    pad_x(2)
    conv1(1, ms=[0])
    wl3_10 = chain_scalar(1, 1, 0)
    wl3_11 = chain_scalar(1, 1, 1)
    chain_vec(1, 1, 0, wl3_10)
    chain_transpose(1, 1, 0)
    chain_vec(1, 1, 1, wl3_11)
    chain_transpose(1, 1, 1)
    bn_math(2)
    pad_x(3)
    conv1(2, ms=[0])
    conv1(0, ms=[1])
    pad_x(4)
    conv1(1, ms=[1])
    quant_absmax(2)
    fold_ivs(2)
    pad_x(5)
    conv1(2, ms=[1])
    # quant chain j=2 while conv1 streams
    wl3_2 = {}
    for c in range(CK):
        for k in range(CK):
            wl3_2[c, k] = chain_scalar(2, c, k)
        for k in range(CK):
            chain_vec(2, c, k, wl3_2[c, k])
            chain_transpose(2, c, k)
        if c == 0:
            conv1(3)
            pad_x(6)
            pad_x(7)
    for i in range(BPC):
        if i + 4 < BPC:
            conv1(i + 4)
        conv2(i)


def build_bass():
    nc = bacc.Bacc(
        "TRN2", target_bir_lowering=False, debug=False, num_devices=NCORES
    )
    td = {}
    td["x"] = nc.dram_tensor("x", (BPC, C, H, W), F32, kind="ExternalInput")
    for j in (1, 2):
        td[f"w{j}"] = nc.dram_tensor(f"w{j}", (C, C, 3, 3), F32, kind="ExternalInput")
        for v in ("gamma", "beta", "mean", "var"):
            td[f"{v}{j}"] = nc.dram_tensor(f"{v}{j}", (C,), F32, kind="ExternalInput")
    td["y"] = nc.dram_tensor("y", (BPC, C, H, W), F32, kind="ExternalOutput")

    with tile.TileContext(nc) as tc:
        with ExitStack() as ctx:
            _emit(nc, tc, ctx, td)
    nc.compile()
    return nc


_NC = None


def _get_nc():
    global _NC
    if _NC is None:
        _NC = build_bass()
    return _NC


def make_in_maps(x, w1, gamma1, beta1, mean1, var1, w2, gamma2, beta2, mean2, var2):
    rep = {
        "w1": w1, "gamma1": gamma1, "beta1": beta1, "mean1": mean1, "var1": var1,
        "w2": w2, "gamma2": gamma2, "beta2": beta2, "mean2": mean2, "var2": var2,
    }
    rep = {k: np.ascontiguousarray(np.asarray(v), dtype=np.float32) for k, v in rep.items()}
    in_maps = []
    for c in range(NCORES):
        m = {"x": np.ascontiguousarray(np.asarray(x)[c * BPC : (c + 1) * BPC], dtype=np.float32)}
        m.update(rep)
        in_maps.append(m)
    return in_maps


def kernel(x, w1, gamma1, beta1, mean1, var1,
           w2, gamma2, beta2, mean2, var2, codebook=None, **_unused):
    nc = _get_nc()
    in_maps = make_in_maps(x, w1, gamma1, beta1, mean1, var1,
                           w2, gamma2, beta2, mean2, var2)
    res = run_bass_kernel_spmd(nc, in_maps, core_ids=list(range(NCORES)))
    return np.concatenate([r["y"] for r in res.results], axis=0)


# revision 23
# speedup vs baseline: 1.0587x; 1.0587x over previous
"""Trainium2 Bass kernel: quantized BasicBlock (quant-conv3x3 -> bn -> relu ->
quant-conv3x3 -> bn -> +residual -> relu).

Sharding: data-parallel over the batch dim of x across 8 NeuronCores (8 images
per core).  Weight quantization (centroid/deviation pipeline) is replicated on
every core, computed on-device.

Math notes:
  - jnp.round (round-half-even) is implemented with the fp32 magic-number
    trick: rne(v) = (v + 1.5*2^23) - 1.5*2^23 for |v| < 2^22.
  - rne(clip(x, -63.5, 63.5)) == clip(rne(x), -64, 64) pointwise, which fuses
    the deviation clamp into two tensor_scalar ops.
  - Quantized weights are integer "levels" dev+cent = k/8 with |k| < 2048,
    exactly representable in fp16.  The global scale `step` is folded into the
    BN scale vector, so matmuls run in fp16 (2x faster than fp32 on the PE)
    with fp32 PSUM accumulation and no weight-precision loss.

Schedule notes (from perfetto traces):
  - The PE queue is strictly in-order; anything emitted ahead of ready work
    stalls it.  BN row DMAs ride the scalar/gpsimd queues so the 16 K=1
    redistribution matmuls (fp16: fp32 PE matmuls trigger a ~7us half-clock
    throttle window) retire in the first ~3us.
  - w1 goes first on the sync ring in 4 descriptors; absmax reduces are
    per-descriptor so istep is ready ~2us after w1 lands.
  - Junk warmup matmuls keep the PE clock pinned at full rate through the
    head's DMA/quant-chain wait.
  - conv matmuls are r-paired (each weight feeds both row chunks) and the
    image-0 m0 conv starts on the k0 tap group while the k1 chain runs.
"""

import sys

for _p in ("/opt/trn_rl_repo",):
    if _p not in sys.path:
        sys.path.insert(0, _p)

from contextlib import ExitStack

import numpy as np

import concourse.tile as tile
from concourse import bacc, mybir
from concourse.bass_utils import run_bass_kernel_spmd
from concourse.masks import make_identity

P = 128
B, C, H, W = 64, 256, 28, 28
NCORES = 8
BPC = B // NCORES          # images per core
CK = C // P                # channel chunks (2)
TAPS = 9
HP, WP = H + 2, W + 2      # zero-padded spatial 30x30
NR = H // 2                # rows per psum chunk (14)
NN = NR * W                # matmul free dim (392)
F32 = mybir.dt.float32
F16 = mybir.dt.float16

MAGIC = 12582912.0         # 1.5 * 2**23  (fp32 RNE round-to-int trick)
HALF_LVLS = 127.0
LV = 8.0                   # 2**(NUM_BITS-1)
CSTEP = HALF_LVLS / LV     # 15.875
DEVW = 64.0                # post-rne clip bound (rne-clip-63.5 fused)
BN_EPS = 1e-5

AF = mybir.ActivationFunctionType
OP = mybir.AluOpType
AX = mybir.AxisListType


def _emit(nc, tc, ctx, td):
    """Emit the whole per-core program.  td: dict of DRAM tensor handles."""
    const = ctx.enter_context(tc.tile_pool(name="const", bufs=1))
    bnp = ctx.enter_context(tc.tile_pool(name="bnp", bufs=2))
    wbig = ctx.enter_context(tc.tile_pool(name="wbig", bufs=1))
    whalf = ctx.enter_context(tc.tile_pool(name="whalf", bufs=4))
    wqp = ctx.enter_context(tc.tile_pool(name="wqp", bufs=1))
    wtp = ctx.enter_context(tc.tile_pool(name="wtp", bufs=1))
    tpq = ctx.enter_context(tc.tile_pool(name="tpq", bufs=1, space="PSUM"))
    tpp = ctx.enter_context(tc.tile_pool(name="tpp", bufs=1, space="PSUM"))
    psp = ctx.enter_context(tc.tile_pool(name="psp", bufs=6, space="PSUM"))
    pxf = ctx.enter_context(tc.tile_pool(name="pxf", bufs=6))
    pxp = ctx.enter_context(tc.tile_pool(name="pxp", bufs=5))
    phh = ctx.enter_context(tc.tile_pool(name="phh", bufs=5))
    pyy = ctx.enter_context(tc.tile_pool(name="pyy", bufs=3))
    pep = ctx.enter_context(tc.tile_pool(name="pep", bufs=3))

    ident16 = const.tile([P, P], F16, name="ident16", tag="ident16")
    make_identity(nc, ident16)
    ident32 = const.tile([P, P], F32, name="ident32", tag="ident32")
    make_identity(nc, ident32)
    ones32 = const.tile([1, P], F32, name="ones32", tag="ones32")
    nc.gpsimd.memset(ones32[:], 1.0)
    magicv = const.tile([P, 1], F32, name="magicv", tag="magicv")
    nc.gpsimd.memset(magicv[:], MAGIC)
    nmagicv = const.tile([P, 1], F32, name="nmagicv", tag="nmagicv")
    nc.gpsimd.memset(nmagicv[:], -MAGIC)
    epsv = const.tile([P, 1], F32, name="epsv", tag="epsv")
    nc.gpsimd.memset(epsv[:], BN_EPS)
    # warm the ScalarE activation tables during the initial DMA wait so the
    # one-time ACT_TABLE_LOADs don't block the quant chain later
    scr = const.tile([P, 1], F32, name="scr", tag="scr")
    nc.scalar.activation(scr[:], magicv[:], AF.Sqrt)
    nc.scalar.activation(scr[:], magicv[:], AF.Relu)
    ones16 = const.tile([1, P], F16, name="ones16", tag="ones16")
    nc.gpsimd.memset(ones16[:], 1.0)
    # warm the DVE with dependency-free ops: several op flavors pay a
    # multi-us first-use cost (observed: 4.9-17us for the first big CAST /
    # tensor_scalar); run one of each during the initial DMA wait instead
    dvew = const.tile([P, 2], F32, name="dvew", tag="dvew")
    nc.vector.memset(dvew[:], 0.0)
    dvew16 = const.tile([P, 2], F16, name="dvew16", tag="dvew16")
    nc.vector.tensor_copy(dvew16[:], dvew[:])

    wT = {}      # wT[j][k] : [P(ci), CK(m), TAPS, P(co)] fp16
    inv_s = {}   # BN scale with quant step folded in: [P, CK]
    bvec = {}    # BN bias: [P, CK]
    _w32 = {}
    _wq = {}
    _istep = {}
    _step = {}
    _inv = {}
    _bnrows = {}
    _psB = {}

    # ---------------- image loads ------------------------------------------
    x_view = td["x"].ap().rearrange("b (c p) h w -> b p c h w", p=P)
    y_view = td["y"].ap().rearrange("b (c p) h w -> b p c h w", p=P)
    xf_t = [None] * BPC
    xp_t = [None] * BPC
    h_t = [None] * BPC

    def load_x_dma(i):
        xf = pxf.tile([P, CK, H, W], F32, name=f"xf{i}", tag="xf")
        nc.sync.dma_start(xf[:], x_view[i])
        xf_t[i] = xf

    def pad_x(i):
        xp = pxp.tile([P, CK, HP, WP], F16, name=f"xp{i}", tag="xp")
        nc.gpsimd.memset(xp[:], 0.0)
        nc.scalar.copy(xp[:, :, 1 : 1 + H, 1 : 1 + W], xf_t[i][:])
        xp_t[i] = xp

    def load_x(i):
        load_x_dma(i)
        pad_x(i)

    # ---------------- per-weight quantization ------------------------------
    def quant_dma(j, nsub=1):
        """Weight DMA: 4*nsub descriptors (per co-chunk x ci-slice) so the
        absmax reduces pipeline with the transfer."""
        w32 = wbig.tile([P, CK, C, TAPS], F32, name=f"w32_{j}", tag="wbig")
        wsrc = td[f"w{j}"].ap().rearrange("(c p) ci kh kw -> p c ci (kh kw)", p=P)
        sz = P // nsub
        for c in range(CK):
            for k in range(CK):
                for q in range(nsub):
                    ks = slice(k * P + q * sz, k * P + (q + 1) * sz)
                    nc.sync.dma_start(w32[:, c, ks, :], wsrc[:, c, ks, :])
        _w32[j] = w32

    def quant_absmax(j):
        """Global absmax -> step/istep (reduces on DVE, combine on PE)."""
        w32 = _w32[j]
        pmq = []
        for c in range(CK):
            for k in range(CK):
                ks = slice(k * P, (k + 1) * P)
                ph = bnp.tile([P, 1], F32, name=f"pmq{j}_{c}_{k}", tag="pmq")
                nc.vector.tensor_reduce(
                    ph[:], w32[:, c, ks, :], axis=AX.XY, op=OP.max,
                    apply_absolute_value=True,
                )
                pmq.append(ph)
        pa = bnp.tile([P, 1], F32, name=f"pa{j}", tag="pa")
        nc.vector.tensor_max(pa[:], pmq[0][:], pmq[1][:])
        pb = bnp.tile([P, 1], F32, name=f"pb{j}", tag="pb")
        nc.vector.tensor_max(pb[:], pmq[2][:], pmq[3][:])
        pm = bnp.tile([P, 1], F32, name=f"pm{j}", tag="pm")
        nc.vector.tensor_max(pm[:], pa[:], pb[:])
        # cross-partition max via PE: transpose [128,1]->[1,128], reduce,
        # then broadcast back with a K=1 ones matmul.  Short fp32 burst only
        # (sustained fp32 PE work triggers a multi-us half-clock window).
        pmt = tpq.tile([1, P], F32, name=f"pmt{j}", tag="tp32")
        nc.tensor.transpose(pmt[:], pm[:], ident32[:])
        sm = bnp.tile([1, 1], F32, name=f"sm{j}", tag="sm")
        nc.vector.tensor_reduce(sm[:], pmt[:], axis=AX.X, op=OP.max)
        pmb = tpq.tile([P, 1], F32, name=f"pmb{j}", tag="tp32")
        nc.tensor.matmul(pmb[:], ones32[:], sm[:])
        pmax = bnp.tile([P, 1], F32, name=f"pmax{j}", tag="pmax")
        nc.vector.tensor_copy(pmax[:], pmb[:])
        step = const.tile([P, 1], F32, name=f"step{j}", tag=f"step{j}")
        nc.vector.tensor_scalar_mul(step[:], pmax[:], 1.0 / HALF_LVLS)
        _step[j] = step
        rmax = bnp.tile([P, 1], F32, name=f"rmax{j}", tag="rmax")
        nc.vector.reciprocal(rmax[:], pmax[:])
        istep = const.tile([P, 1], F32, name=f"istep{j}", tag=f"istep{j}")
        nc.vector.tensor_scalar_mul(istep[:], rmax[:], HALF_LVLS)
        _istep[j] = istep
        if j == 1:
            i16 = const.tile([P, 1], F16, name="istep16", tag="istep16")
            nc.vector.tensor_copy(i16[:], istep[:])
            _istep16[0] = i16

        wq = wqp.tile([P, CK, C, TAPS], F16, name=f"wq{j}", tag=f"wq{j}")
        _wq[j] = wq
        wT[j] = []
        for k in range(CK):
            wt = wtp.tile([P, CK, TAPS, P], F16, name=f"wT{j}_{k}", tag=f"wT{j}_{k}")
            wT[j].append(wt)

    def fold_ivs(j):
        """inv_s = inv * step (after both bn_math(j) and quant_absmax(j))."""
        ivs = const.tile([P, CK], F32, name=f"ivs{j}", tag=f"ivs{j}")
        nc.vector.tensor_scalar_mul(ivs[:], _inv[j][:], _step[j][:, 0:1])
        inv_s[j] = ivs

    def bn_prep(j):
        """BN row DMAs (sync ring, behind w1) + fp16 casts on DVE."""
        rows = {}
        for v in ("gamma", "beta", "mean", "var"):
            rv = bnp.tile([1, C], F32, name=f"{v}{j}", tag=f"{v}{j}")
            nc.sync.dma_start(rv[:], td[f"{v}{j}"].ap().unsqueeze(0))
            r16 = bnp.tile([1, C], F16, name=f"{v}16_{j}", tag=f"{v}16_{j}")
            nc.scalar.copy(r16[:], rv[:])
            rows[v] = r16
        _bnrows[j] = rows

    def bn_mms(j):
        """K=1 fp16 redistribution matmuls: [1,256] rows -> [128, 2]."""
        rows = _bnrows[j]
        psB = tpq.tile([P, 4 * CK], F32, name=f"psB{j}", tag="tp32")
        for v, key in enumerate(("gamma", "beta", "mean", "var")):
            row = rows[key]
            for c in range(CK):
                nc.tensor.matmul(
                    psB[:, v * CK + c : v * CK + c + 1],
                    row[0:1, c * P : (c + 1) * P],
                    ones16[0:1, 0:1],
                )
        _psB[j] = psB

    def bn_math(j):
        psB = _psB[j]
        bn4 = bnp.tile([P, 4, CK], F32, name=f"bn4_{j}", tag=f"bn4_{j}")
        nc.vector.tensor_copy(bn4[:], psB[:].rearrange("p (v c) -> p v c", c=CK))
        gvp, bevp, muvp, vavp = (bn4[:, v, :] for v in range(4))
        tv = bnp.tile([P, CK], F32, name=f"tv{j}", tag="btmp")
        nc.scalar.activation(tv[:], vavp, AF.Identity, bias=epsv[:, 0:1])
        rv = bnp.tile([P, CK], F32, name=f"rv{j}", tag="btmp")
        nc.vector.reciprocal(rv[:], tv[:])
        sv = bnp.tile([P, CK], F32, name=f"sv{j}", tag="btmp")
        nc.scalar.activation(sv[:], rv[:], AF.Sqrt)           # rsqrt(var+eps)
        inv = const.tile([P, CK], F32, name=f"inv{j}", tag=f"inv{j}")
        nc.vector.tensor_mul(inv[:], sv[:], gvp)              # gamma * rsqrt
        mi = bnp.tile([P, CK], F32, name=f"mi{j}", tag="btmp")
        nc.vector.tensor_mul(mi[:], muvp, inv[:])
        bv = const.tile([P, CK], F32, name=f"bv{j}", tag=f"bv{j}")
        nc.vector.tensor_sub(bv[:], bevp, mi[:])              # beta - mean*inv
        bvec[j] = bv
        _inv[j] = inv

    def chain_scalar(j, c, k):
        """Scalar-engine part of the quant chain for chunk (c, k):
        wl3 = rne(w * istep) - MAGIC.  The +-127 clip is redundant:
        |w*istep| <= 127*(1+2^-23) by construction, and rne of that is 127."""
        w32, istep = _w32[j], _istep[j]
        ks = slice(k * P, (k + 1) * P)
        wlr = whalf.tile([P, P, TAPS], F32, name=f"wlr{j}_{c}_{k}", tag="wh")
        nc.scalar.activation(
            wlr[:], w32[:, c, ks, :], AF.Identity,
            bias=magicv[:, 0:1], scale=istep[:, 0:1],
        )
        return wlr

    def chain_vec(j, c, k, wlr):
        """DVE part: wl3 -> grain means -> centroid -> fused clip/rne -> wq.
        (The centroid's +-8 clip is also redundant: |gm|/9/cstep <= 8.)"""
        wq = _wq[j]
        ks = slice(k * P, (k + 1) * P)
        wl3 = whalf.tile([P, P, TAPS], F32, name=f"wl3{j}_{c}_{k}", tag="wh")
        nc.vector.tensor_scalar_sub(wl3[:], wlr[:], MAGIC)
        gm = bnp.tile([P, P], F32, name=f"gm{j}_{c}_{k}", tag="gm")
        nc.vector.tensor_reduce(gm[:], wl3[:], axis=AX.X, op=OP.add)
        c1 = bnp.tile([P, P], F32, name=f"c1{j}_{c}_{k}", tag="c1")
        nc.vector.tensor_scalar(
            c1[:], gm[:], 1.0 / (TAPS * CSTEP), MAGIC, OP.mult, OP.add
        )
        cent = bnp.tile([P, P], F32, name=f"cent{j}_{c}_{k}", tag="cent")
        nc.vector.tensor_scalar(cent[:], c1[:], MAGIC, CSTEP, OP.subtract, OP.mult)
        mc = bnp.tile([P, P], F32, name=f"mc{j}_{c}_{k}", tag="mc")
        nc.vector.tensor_scalar(mc[:], cent[:], -1.0, MAGIC, OP.mult, OP.add)
        centb = cent.unsqueeze(2).broadcast_to((P, P, TAPS))
        mcb = mc.unsqueeze(2).broadcast_to((P, P, TAPS))
        t1 = whalf.tile([P, P, TAPS], F32, name=f"t1{j}_{c}_{k}", tag="wh")
        nc.vector.tensor_add(t1[:], wl3[:], mcb)          # wl3 + MAGIC - cent
        t2 = whalf.tile([P, P, TAPS], F32, name=f"t2{j}_{c}_{k}", tag="wh")
        nc.vector.tensor_scalar(t2[:], t1[:], MAGIC, DEVW, OP.subtract, OP.min)
        # wq = max(t2, -64) + cent   (fp16-exact: |8*wq| <= 1528 < 2048)
        nc.vector.scalar_tensor_tensor(
            wq[:, c, ks, :], t2[:], -DEVW, centb, OP.max, OP.add
        )

    def chain_transpose(j, c, k):
        """PE-transpose the 9 taps of chunk (m=c, k): [co,ci] -> [ci,co]."""
        wq = _wq[j]
        for t0 in (0, 4, 8):
            nb = min(4, TAPS - t0)
            pst = tpp.tile([P, nb, P], F16, name=f"pst{j}_{c}_{k}_{t0}", tag="tp")
            for dt in range(nb):
                nc.tensor.transpose(
                    pst[:, dt, :],
                    wq[:, c, k * P : (k + 1) * P, t0 + dt],
                    ident16[:],
                )
            nc.scalar.copy(wT[j][k][:, c, t0 : t0 + nb, :], pst[:])

    # ---------------- convolutions -----------------------------------------
    def conv_mms(ps_r, src16, wTj, m, ks=None):
        """Accumulate the 3x3xC conv into the psum chunks in ps_r (keyed by
        r).  Contiguous 9-tap blocks per bank (interleaving banks MM-by-MM
        costs ~35ns/MM); k-major so a k-half can start before the other
        half's weights exist."""
        klist = list(range(CK)) if ks is None else list(ks)
        for k in klist:
            for r in ps_r:
                r0 = r * NR
                for t in range(TAPS):
                    dh, dw = t // 3, t % 3
                    nc.tensor.matmul(
                        ps_r[r][:],
                        wTj[k][:, m, t, :],
                        src16[:, k, r0 + dh : r0 + dh + NR, dw : dw + W],
                        start=(k == 0 and t == 0),
                        stop=(k == CK - 1 and t == TAPS - 1),
                    )

    def conv1(i, ms=None, ks=None, ps_store=None):
        if ms is None or ms == [0]:
            hh = phh.tile([P, CK, HP, WP], F16, name=f"h{i}", tag="h")
            nc.gpsimd.memset(hh[:], 0.0)
            h_t[i] = hh
        hh = h_t[i]
        for m in (ms if ms is not None else range(CK)):
            if ks is None or 0 in ks:
                ps_r = {r: psp.tile([P, NN], F32, name=f"ps1_{i}_{m}_{r}", tag="ps")
                        for r in range(2)}
                if ps_store is not None:
                    ps_store[i, m] = ps_r
            else:
                ps_r = ps_store[i, m]
            conv_mms(ps_r, xp_t[i], wT[1], m, ks=ks)
            if ks is not None and CK - 1 not in ks:
                continue
            for r in range(2):
                r0 = r * NR
                nc.scalar.activation(
                    hh[:, m, 1 + r0 : 1 + r0 + NR, 1 : 1 + W],
                    ps_r[r].rearrange("p (r w) -> p r w", w=W),
                    AF.Relu,
                    bias=bvec[1][:, m : m + 1],
                    scale=inv_s[1][:, m : m + 1],
                )

    def conv2_fine(i, m):
        """Last chunk: 4 quarter-height psums so the final epilogue chain is
        short and the tail drains fast."""
        yf = pyy.tile([P, H, W], F32, name=f"y{i}_{m}", tag="y")
        NQ = NR // 2                  # 7 rows
        ps_q = {q: psp.tile([P, NQ * W], F32, name=f"psq_{i}_{m}_{q}", tag="ps")
                for q in range(4)}
        for k in range(CK):
            for q in range(4):
                q0 = q * NQ
                for t in range(TAPS):
                    dh, dw = t // 3, t % 3
                    nc.tensor.matmul(
                        ps_q[q][:],
                        wT[2][k][:, m, t, :],
                        h_t[i][:, k, q0 + dh : q0 + dh + NQ, dw : dw + W],
                        start=(k == 0 and t == 0),
                        stop=(k == CK - 1 and t == TAPS - 1),
                    )
        for q in range(4):
            q0 = q * NQ
            t2 = pep.tile([P, NQ * W], F32, name=f"t2q_{i}_{m}_{q}", tag="t2")
            nc.scalar.activation(
                t2[:], ps_q[q][:], AF.Identity,
                bias=bvec[2][:, m : m + 1], scale=inv_s[2][:, m : m + 1],
            )
            u = pep.tile([P, NQ * W], F32, name=f"uq_{i}_{m}_{q}", tag="u")
            xflat = xf_t[i][:, m, q0 : q0 + NQ, :].rearrange("p r w -> p (r w)")
            nc.vector.tensor_add(u[:], t2[:], xflat)
            nc.scalar.activation(
                yf[:, q0 : q0 + NQ, :], u.rearrange("p (r w) -> p r w", w=W),
                AF.Relu,
            )
            nc.sync.dma_start(y_view[i][:, m, q0 : q0 + NQ, :], yf[:, q0 : q0 + NQ, :])

    def conv2(i):
        for m in range(CK):
            if i == BPC - 1 and m == CK - 1:
                conv2_fine(i, m)
                continue
            yf = pyy.tile([P, H, W], F32, name=f"y{i}_{m}", tag="y")
            ps_r = {r: psp.tile([P, NN], F32, name=f"ps2_{i}_{m}_{r}", tag="ps")
                    for r in range(2)}
            conv_mms(ps_r, h_t[i], wT[2], m)
            for r in range(2):
                r0 = r * NR
                t2 = pep.tile([P, NN], F32, name=f"t2_{i}_{m}_{r}", tag="t2")
                nc.scalar.activation(
                    t2[:],
                    ps_r[r][:],
                    AF.Identity,
                    bias=bvec[2][:, m : m + 1],
                    scale=inv_s[2][:, m : m + 1],
                )
                u = pep.tile([P, NN], F32, name=f"u_{i}_{m}_{r}", tag="u")
                xflat = xf_t[i][:, m, r0 : r0 + NR, :].rearrange("p r w -> p (r w)")
                nc.vector.tensor_add(u[:], t2[:], xflat)
                nc.scalar.activation(
                    yf[:, r0 : r0 + NR, :],
                    u.rearrange("p (r w) -> p r w", w=W),
                    AF.Relu,
                )
                if i >= BPC - 2:
                    nc.sync.dma_start(
                        y_view[i][:, m, r0 : r0 + NR, :], yf[:, r0 : r0 + NR, :]
                    )
            if i < BPC - 2:
                nc.sync.dma_start(y_view[i][:, m], yf[:])

    def pe_warmup(n, gated=False):
        """Junk matmuls to hold the PE clock at full rate through the head's
        DMA wait, so real matmuls start warm.  gated=True makes them depend
        on istep so the scheduler cannot run them before the absmax path."""
        for i in range(n):
            scr_ps = psp.tile([P, NN], F32, name=f"warm{_wuid[0]}", tag="ps")
            _wuid[0] += 1
            if gated:
                rhsb = _istep16[0].broadcast_to((P, 3 * P))
            else:
                rhsb = ident16.unsqueeze(1).broadcast_to((P, 3, P))
            nc.tensor.matmul(scr_ps[:, 0 : 3 * P], ident16[:], rhsb)

    _wuid = [0]
    _istep16 = [None]

    # ---------------- emission order (engine priority) ---------------------
    # sync ring: w1 (4 descs) -> x0 -> x1 -> w2 -> x2..x7
    # scalar queue: bn1 rows; gpsimd queue: bn2 rows.
    quant_dma(1, nsub=2)
    bn_prep(1)
    bn_prep(2)
    load_x_dma(0)
    load_x_dma(1)
    quant_dma(2)
    for i in range(2, BPC):
        load_x_dma(i)

    pe_warmup(46)
    quant_absmax(1)          # DVE: reduces..istep + istep16; PE: T + bcast
    bn_mms(1)                # PE ~20.5us; rows land ~18.4 behind w1
    bn_mms(2)
    bn_math(1)
    fold_ivs(1)
    pe_warmup(64, gated=True)
    # chain c0: scalar ops for both k first, then DVE chains; transpose k0
    # early so image-0 m0 k0 taps start while the k1 chain runs on DVE.
    wl3_00 = chain_scalar(1, 0, 0)
    wl3_01 = chain_scalar(1, 0, 1)
    pad_x(0)
    pad_x(1)
    chain_vec(1, 0, 0, wl3_00)
    chain_transpose(1, 0, 0)
    ps_store = {}
    conv1(0, ms=[0], ks=[0], ps_store=ps_store)
    pe_warmup(12, gated=True)
    chain_vec(1, 0, 1, wl3_01)
    chain_transpose(1, 0, 1)
    conv1(0, ms=[0], ks=[1], ps_store=ps_store)
    pad_x(2)
    conv1(1, ms=[0])
    wl3_10 = chain_scalar(1, 1, 0)
    wl3_11 = chain_scalar(1, 1, 1)
    chain_vec(1, 1, 0, wl3_10)
    chain_transpose(1, 1, 0)
    chain_vec(1, 1, 1, wl3_11)
    chain_transpose(1, 1, 1)
    bn_math(2)
    pad_x(3)
    conv1(2, ms=[0])
    conv1(0, ms=[1])
    pad_x(4)
    conv1(1, ms=[1])
    quant_absmax(2)
    fold_ivs(2)
    pad_x(5)
    conv1(2, ms=[1])
    # quant chain j=2 while conv1 streams
    wl3_2 = {}
    for c in range(CK):
        for k in range(CK):
            wl3_2[c, k] = chain_scalar(2, c, k)
        for k in range(CK):
            chain_vec(2, c, k, wl3_2[c, k])
            chain_transpose(2, c, k)
        if c == 0:
            conv1(3)
            pad_x(6)
            pad_x(7)
    for i in range(BPC):
        if i + 4 < BPC:
            conv1(i + 4)
        conv2(i)


def build_bass():
    nc = bacc.Bacc(
        "TRN2", target_bir_lowering=False, debug=False, num_devices=NCORES
    )
    td = {}
    td["x"] = nc.dram_tensor("x", (BPC, C, H, W), F32, kind="ExternalInput")
    for j in (1, 2):
        td[f"w{j}"] = nc.dram_tensor(f"w{j}", (C, C, 3, 3), F32, kind="ExternalInput")
        for v in ("gamma", "beta", "mean", "var"):
            td[f"{v}{j}"] = nc.dram_tensor(f"{v}{j}", (C,), F32, kind="ExternalInput")
    td["y"] = nc.dram_tensor("y", (BPC, C, H, W), F32, kind="ExternalOutput")

    with tile.TileContext(nc) as tc:
        with ExitStack() as ctx:
            _emit(nc, tc, ctx, td)
    nc.compile()
    return nc


_NC = None


def _get_nc():
    global _NC
    if _NC is None:
        _NC = build_bass()
    return _NC


def make_in_maps(x, w1, gamma1, beta1, mean1, var1, w2, gamma2, beta2, mean2, var2):
    rep = {
        "w1": w1, "gamma1": gamma1, "beta1": beta1, "mean1": mean1, "var1": var1,
        "w2": w2, "gamma2": gamma2, "beta2": beta2, "mean2": mean2, "var2": var2,
    }
    rep = {k: np.ascontiguousarray(np.asarray(v), dtype=np.float32) for k, v in rep.items()}
    in_maps = []
    for c in range(NCORES):
        m = {"x": np.ascontiguousarray(np.asarray(x)[c * BPC : (c + 1) * BPC], dtype=np.float32)}
        m.update(rep)
        in_maps.append(m)
    return in_maps


def kernel(x, w1, gamma1, beta1, mean1, var1,
           w2, gamma2, beta2, mean2, var2, codebook=None, **_unused):
    nc = _get_nc()
    in_maps = make_in_maps(x, w1, gamma1, beta1, mean1, var1,
                           w2, gamma2, beta2, mean2, var2)
    res = run_bass_kernel_spmd(nc, in_maps, core_ids=list(range(NCORES)))
    return np.concatenate([r["y"] for r in res.results], axis=0)


# revision 24
# speedup vs baseline: 1.0638x; 1.0048x over previous
"""Trainium2 Bass kernel: quantized BasicBlock (quant-conv3x3 -> bn -> relu ->
quant-conv3x3 -> bn -> +residual -> relu).

Sharding: data-parallel over the batch dim of x across 8 NeuronCores (8 images
per core).  Weight quantization (centroid/deviation pipeline) is replicated on
every core, computed on-device.

Math notes:
  - jnp.round (round-half-even) is implemented with the fp32 magic-number
    trick: rne(v) = (v + 1.5*2^23) - 1.5*2^23 for |v| < 2^22.
  - rne(clip(x, -63.5, 63.5)) == clip(rne(x), -64, 64) pointwise, which fuses
    the deviation clamp into two tensor_scalar ops.
  - Quantized weights are integer "levels" dev+cent = k/8 with |k| < 2048,
    exactly representable in fp16.  The global scale `step` is folded into the
    BN scale vector, so matmuls run in fp16 (2x faster than fp32 on the PE)
    with fp32 PSUM accumulation and no weight-precision loss.

Schedule notes (from perfetto traces):
  - The PE queue is strictly in-order; anything emitted ahead of ready work
    stalls it.  BN row DMAs ride the scalar/gpsimd queues so the 16 K=1
    redistribution matmuls (fp16: fp32 PE matmuls trigger a ~7us half-clock
    throttle window) retire in the first ~3us.
  - w1 goes first on the sync ring in 4 descriptors; absmax reduces are
    per-descriptor so istep is ready ~2us after w1 lands.
  - Junk warmup matmuls keep the PE clock pinned at full rate through the
    head's DMA/quant-chain wait.
  - conv matmuls are r-paired (each weight feeds both row chunks) and the
    image-0 m0 conv starts on the k0 tap group while the k1 chain runs.
"""

import sys

for _p in ("/opt/trn_rl_repo",):
    if _p not in sys.path:
        sys.path.insert(0, _p)

from contextlib import ExitStack

import numpy as np

import concourse.tile as tile
from concourse import bacc, mybir
from concourse.bass_utils import run_bass_kernel_spmd
from concourse.masks import make_identity

P = 128
B, C, H, W = 64, 256, 28, 28
NCORES = 8
BPC = B // NCORES          # images per core
CK = C // P                # channel chunks (2)
TAPS = 9
HP, WP = H + 2, W + 2      # zero-padded spatial 30x30
NR = H // 2                # rows per psum chunk (14)
NN = NR * W                # matmul free dim (392)
F32 = mybir.dt.float32
F16 = mybir.dt.float16

MAGIC = 12582912.0         # 1.5 * 2**23  (fp32 RNE round-to-int trick)
HALF_LVLS = 127.0
LV = 8.0                   # 2**(NUM_BITS-1)
CSTEP = HALF_LVLS / LV     # 15.875
DEVW = 64.0                # post-rne clip bound (rne-clip-63.5 fused)
BN_EPS = 1e-5

AF = mybir.ActivationFunctionType
OP = mybir.AluOpType
AX = mybir.AxisListType


def _emit(nc, tc, ctx, td):
    """Emit the whole per-core program.  td: dict of DRAM tensor handles."""
    const = ctx.enter_context(tc.tile_pool(name="const", bufs=1))
    bnp = ctx.enter_context(tc.tile_pool(name="bnp", bufs=2))
    wbig = ctx.enter_context(tc.tile_pool(name="wbig", bufs=1))
    whalf = ctx.enter_context(tc.tile_pool(name="whalf", bufs=4))
    wqp = ctx.enter_context(tc.tile_pool(name="wqp", bufs=1))
    wtp = ctx.enter_context(tc.tile_pool(name="wtp", bufs=1))
    tpq = ctx.enter_context(tc.tile_pool(name="tpq", bufs=1, space="PSUM"))
    tpp = ctx.enter_context(tc.tile_pool(name="tpp", bufs=1, space="PSUM"))
    psp = ctx.enter_context(tc.tile_pool(name="psp", bufs=6, space="PSUM"))
    pxf = ctx.enter_context(tc.tile_pool(name="pxf", bufs=6))
    pxp = ctx.enter_context(tc.tile_pool(name="pxp", bufs=5))
    phh = ctx.enter_context(tc.tile_pool(name="phh", bufs=5))
    pyy = ctx.enter_context(tc.tile_pool(name="pyy", bufs=3))
    pep = ctx.enter_context(tc.tile_pool(name="pep", bufs=3))

    ident16 = const.tile([P, P], F16, name="ident16", tag="ident16")
    make_identity(nc, ident16)
    ident32 = const.tile([P, P], F32, name="ident32", tag="ident32")
    make_identity(nc, ident32)
    ones32 = const.tile([1, P], F32, name="ones32", tag="ones32")
    nc.gpsimd.memset(ones32[:], 1.0)
    magicv = const.tile([P, 1], F32, name="magicv", tag="magicv")
    nc.gpsimd.memset(magicv[:], MAGIC)
    nmagicv = const.tile([P, 1], F32, name="nmagicv", tag="nmagicv")
    nc.gpsimd.memset(nmagicv[:], -MAGIC)
    epsv = const.tile([P, 1], F32, name="epsv", tag="epsv")
    nc.gpsimd.memset(epsv[:], BN_EPS)
    # warm the ScalarE activation tables during the initial DMA wait so the
    # one-time ACT_TABLE_LOADs don't block the quant chain later
    scr = const.tile([P, 1], F32, name="scr", tag="scr")
    nc.scalar.activation(scr[:], magicv[:], AF.Sqrt)
    nc.scalar.activation(scr[:], magicv[:], AF.Relu)
    ones16 = const.tile([1, P], F16, name="ones16", tag="ones16")
    nc.gpsimd.memset(ones16[:], 1.0)
    # warm the DVE with dependency-free ops: several op flavors pay a
    # multi-us first-use cost (observed: 4.9-17us for the first big CAST /
    # tensor_scalar); run one of each during the initial DMA wait instead
    dvew = const.tile([P, 2], F32, name="dvew", tag="dvew")
    nc.vector.memset(dvew[:], 0.0)
    dvew16 = const.tile([P, 2], F16, name="dvew16", tag="dvew16")
    nc.vector.tensor_copy(dvew16[:], dvew[:])

    wT = {}      # wT[j][k] : [P(ci), CK(m), TAPS, P(co)] fp16
    inv_s = {}   # BN scale with quant step folded in: [P, CK]
    bvec = {}    # BN bias: [P, CK]
    _w32 = {}
    _wq = {}
    _istep = {}
    _step = {}
    _inv = {}
    _bnrows = {}
    _psB = {}

    # ---------------- image loads ------------------------------------------
    x_view = td["x"].ap().rearrange("b (c p) h w -> b p c h w", p=P)
    y_view = td["y"].ap().rearrange("b (c p) h w -> b p c h w", p=P)
    xf_t = [None] * BPC
    xp_t = [None] * BPC
    h_t = [None] * BPC

    def load_x_dma(i):
        xf = pxf.tile([P, CK, H, W], F32, name=f"xf{i}", tag="xf")
        nc.sync.dma_start(xf[:], x_view[i])
        xf_t[i] = xf

    def pad_x(i):
        xp = pxp.tile([P, CK, HP, WP], F16, name=f"xp{i}", tag="xp")
        nc.gpsimd.memset(xp[:], 0.0)
        nc.scalar.copy(xp[:, :, 1 : 1 + H, 1 : 1 + W], xf_t[i][:])
        xp_t[i] = xp

    def load_x(i):
        load_x_dma(i)
        pad_x(i)

    # ---------------- per-weight quantization ------------------------------
    def quant_dma(j, nsub=1):
        """Weight DMA: 4*nsub descriptors (per co-chunk x ci-slice) so the
        absmax reduces pipeline with the transfer."""
        w32 = wbig.tile([P, CK, C, TAPS], F32, name=f"w32_{j}", tag="wbig")
        wsrc = td[f"w{j}"].ap().rearrange("(c p) ci kh kw -> p c ci (kh kw)", p=P)
        sz = P // nsub
        for c in range(CK):
            for k in range(CK):
                for q in range(nsub):
                    ks = slice(k * P + q * sz, k * P + (q + 1) * sz)
                    nc.sync.dma_start(w32[:, c, ks, :], wsrc[:, c, ks, :])
        _w32[j] = w32

    def quant_absmax(j):
        """Global absmax -> step/istep (reduces on DVE, combine on PE)."""
        w32 = _w32[j]
        pmq = []
        for c in range(CK):
            for k in range(CK):
                ks = slice(k * P, (k + 1) * P)
                ph = bnp.tile([P, 1], F32, name=f"pmq{j}_{c}_{k}", tag="pmq")
                nc.vector.tensor_reduce(
                    ph[:], w32[:, c, ks, :], axis=AX.XY, op=OP.max,
                    apply_absolute_value=True,
                )
                pmq.append(ph)
        pa = bnp.tile([P, 1], F32, name=f"pa{j}", tag="pa")
        nc.vector.tensor_max(pa[:], pmq[0][:], pmq[1][:])
        pb = bnp.tile([P, 1], F32, name=f"pb{j}", tag="pb")
        nc.vector.tensor_max(pb[:], pmq[2][:], pmq[3][:])
        pm = bnp.tile([P, 1], F32, name=f"pm{j}", tag="pm")
        nc.vector.tensor_max(pm[:], pa[:], pb[:])
        # cross-partition max via PE: transpose [128,1]->[1,128], reduce,
        # then broadcast back with a K=1 ones matmul.  Short fp32 burst only
        # (sustained fp32 PE work triggers a multi-us half-clock window).
        pmt = tpq.tile([1, P], F32, name=f"pmt{j}", tag="tp32")
        nc.tensor.transpose(pmt[:], pm[:], ident32[:])
        sm = bnp.tile([1, 1], F32, name=f"sm{j}", tag="sm")
        nc.vector.tensor_reduce(sm[:], pmt[:], axis=AX.X, op=OP.max)
        pmb = tpq.tile([P, 1], F32, name=f"pmb{j}", tag="tp32")
        nc.tensor.matmul(pmb[:], ones32[:], sm[:])
        pmax = bnp.tile([P, 1], F32, name=f"pmax{j}", tag="pmax")
        nc.vector.tensor_copy(pmax[:], pmb[:])
        step = const.tile([P, 1], F32, name=f"step{j}", tag=f"step{j}")
        nc.vector.tensor_scalar_mul(step[:], pmax[:], 1.0 / HALF_LVLS)
        _step[j] = step
        rmax = bnp.tile([P, 1], F32, name=f"rmax{j}", tag="rmax")
        nc.vector.reciprocal(rmax[:], pmax[:])
        istep = const.tile([P, 1], F32, name=f"istep{j}", tag=f"istep{j}")
        nc.vector.tensor_scalar_mul(istep[:], rmax[:], HALF_LVLS)
        _istep[j] = istep
        if j == 1:
            i16 = const.tile([P, 1], F16, name="istep16", tag="istep16")
            nc.vector.tensor_copy(i16[:], istep[:])
            _istep16[0] = i16

        wq = wqp.tile([P, CK, C, TAPS], F16, name=f"wq{j}", tag=f"wq{j}")
        _wq[j] = wq
        wT[j] = []
        for k in range(CK):
            wt = wtp.tile([P, CK, TAPS, P], F16, name=f"wT{j}_{k}", tag=f"wT{j}_{k}")
            wT[j].append(wt)

    def fold_ivs(j):
        """inv_s = inv * step (after both bn_math(j) and quant_absmax(j))."""
        ivs = const.tile([P, CK], F32, name=f"ivs{j}", tag=f"ivs{j}")
        nc.vector.tensor_scalar_mul(ivs[:], _inv[j][:], _step[j][:, 0:1])
        inv_s[j] = ivs

    def bn_prep(j):
        """BN row DMAs (sync ring, behind w1) + fp16 casts on DVE."""
        rows = {}
        for v in ("gamma", "beta", "mean", "var"):
            rv = bnp.tile([1, C], F32, name=f"{v}{j}", tag=f"{v}{j}")
            nc.sync.dma_start(rv[:], td[f"{v}{j}"].ap().unsqueeze(0))
            r16 = bnp.tile([1, C], F16, name=f"{v}16_{j}", tag=f"{v}16_{j}")
            nc.scalar.copy(r16[:], rv[:])
            rows[v] = r16
        _bnrows[j] = rows

    def bn_mms(j):
        """K=1 fp16 redistribution matmuls: [1,256] rows -> [128, 2]."""
        rows = _bnrows[j]
        psB = tpq.tile([P, 4 * CK], F32, name=f"psB{j}", tag="tp32")
        for v, key in enumerate(("gamma", "beta", "mean", "var")):
            row = rows[key]
            for c in range(CK):
                nc.tensor.matmul(
                    psB[:, v * CK + c : v * CK + c + 1],
                    row[0:1, c * P : (c + 1) * P],
                    ones16[0:1, 0:1],
                )
        _psB[j] = psB

    def bn_math(j):
        psB = _psB[j]
        bn4 = bnp.tile([P, 4, CK], F32, name=f"bn4_{j}", tag=f"bn4_{j}")
        nc.vector.tensor_copy(bn4[:], psB[:].rearrange("p (v c) -> p v c", c=CK))
        gvp, bevp, muvp, vavp = (bn4[:, v, :] for v in range(4))
        tv = bnp.tile([P, CK], F32, name=f"tv{j}", tag="btmp")
        nc.scalar.activation(tv[:], vavp, AF.Identity, bias=epsv[:, 0:1])
        rv = bnp.tile([P, CK], F32, name=f"rv{j}", tag="btmp")
        nc.vector.reciprocal(rv[:], tv[:])
        sv = bnp.tile([P, CK], F32, name=f"sv{j}", tag="btmp")
        nc.scalar.activation(sv[:], rv[:], AF.Sqrt)           # rsqrt(var+eps)
        inv = const.tile([P, CK], F32, name=f"inv{j}", tag=f"inv{j}")
        nc.vector.tensor_mul(inv[:], sv[:], gvp)              # gamma * rsqrt
        mi = bnp.tile([P, CK], F32, name=f"mi{j}", tag="btmp")
        nc.vector.tensor_mul(mi[:], muvp, inv[:])
        bv = const.tile([P, CK], F32, name=f"bv{j}", tag=f"bv{j}")
        nc.vector.tensor_sub(bv[:], bevp, mi[:])              # beta - mean*inv
        bvec[j] = bv
        _inv[j] = inv

    def chain_scalar(j, c, k):
        """Scalar-engine part of the quant chain for chunk (c, k):
        wl3 = rne(w * istep) - MAGIC.  The +-127 clip is redundant:
        |w*istep| <= 127*(1+2^-23) by construction, and rne of that is 127."""
        w32, istep = _w32[j], _istep[j]
        ks = slice(k * P, (k + 1) * P)
        wlr = whalf.tile([P, P, TAPS], F32, name=f"wlr{j}_{c}_{k}", tag="wh")
        nc.scalar.activation(
            wlr[:], w32[:, c, ks, :], AF.Identity,
            bias=magicv[:, 0:1], scale=istep[:, 0:1],
        )
        return wlr

    def chain_vec(j, c, k, wlr):
        """DVE part: wl3 -> grain means -> centroid -> fused clip/rne -> wq.
        (The centroid's +-8 clip is also redundant: |gm|/9/cstep <= 8.)"""
        wq = _wq[j]
        ks = slice(k * P, (k + 1) * P)
        wl3 = whalf.tile([P, P, TAPS], F32, name=f"wl3{j}_{c}_{k}", tag="wh")
        nc.vector.tensor_scalar_sub(wl3[:], wlr[:], MAGIC)
        gm = bnp.tile([P, P], F32, name=f"gm{j}_{c}_{k}", tag="gm")
        nc.vector.tensor_reduce(gm[:], wl3[:], axis=AX.X, op=OP.add)
        c1 = bnp.tile([P, P], F32, name=f"c1{j}_{c}_{k}", tag="c1")
        nc.vector.tensor_scalar(
            c1[:], gm[:], 1.0 / (TAPS * CSTEP), MAGIC, OP.mult, OP.add
        )
        cent = bnp.tile([P, P], F32, name=f"cent{j}_{c}_{k}", tag="cent")
        nc.vector.tensor_scalar(cent[:], c1[:], MAGIC, CSTEP, OP.subtract, OP.mult)
        mc = bnp.tile([P, P], F32, name=f"mc{j}_{c}_{k}", tag="mc")
        nc.vector.tensor_scalar(mc[:], cent[:], -1.0, MAGIC, OP.mult, OP.add)
        centb = cent.unsqueeze(2).broadcast_to((P, P, TAPS))
        mcb = mc.unsqueeze(2).broadcast_to((P, P, TAPS))
        t1 = whalf.tile([P, P, TAPS], F32, name=f"t1{j}_{c}_{k}", tag="wh")
        nc.vector.tensor_add(t1[:], wl3[:], mcb)          # wl3 + MAGIC - cent
        t2 = whalf.tile([P, P, TAPS], F32, name=f"t2{j}_{c}_{k}", tag="wh")
        nc.vector.tensor_scalar(t2[:], t1[:], MAGIC, DEVW, OP.subtract, OP.min)
        # wq = max(t2, -64) + cent   (fp16-exact: |8*wq| <= 1528 < 2048)
        nc.vector.scalar_tensor_tensor(
            wq[:, c, ks, :], t2[:], -DEVW, centb, OP.max, OP.add
        )

    def chain_transpose(j, c, k):
        """PE-transpose the 9 taps of chunk (m=c, k): [co,ci] -> [ci,co]."""
        wq = _wq[j]
        for t0 in (0, 4, 8):
            nb = min(4, TAPS - t0)
            pst = tpp.tile([P, nb, P], F16, name=f"pst{j}_{c}_{k}_{t0}", tag="tp")
            for dt in range(nb):
                nc.tensor.transpose(
                    pst[:, dt, :],
                    wq[:, c, k * P : (k + 1) * P, t0 + dt],
                    ident16[:],
                )
            nc.scalar.copy(wT[j][k][:, c, t0 : t0 + nb, :], pst[:])

    # ---------------- convolutions -----------------------------------------
    def conv_mms(ps_r, src16, wTj, m, ks=None):
        """Accumulate the 3x3xC conv into the psum chunks in ps_r (keyed by
        r).  Contiguous 9-tap blocks per bank (interleaving banks MM-by-MM
        costs ~35ns/MM); k-major so a k-half can start before the other
        half's weights exist."""
        klist = list(range(CK)) if ks is None else list(ks)
        for k in klist:
            for r in ps_r:
                r0 = r * NR
                for t in range(TAPS):
                    dh, dw = t // 3, t % 3
                    nc.tensor.matmul(
                        ps_r[r][:],
                        wTj[k][:, m, t, :],
                        src16[:, k, r0 + dh : r0 + dh + NR, dw : dw + W],
                        start=(k == 0 and t == 0),
                        stop=(k == CK - 1 and t == TAPS - 1),
                    )

    def conv1(i, ms=None, ks=None, ps_store=None):
        if ms is None or ms == [0]:
            hh = phh.tile([P, CK, HP, WP], F16, name=f"h{i}", tag="h")
            nc.gpsimd.memset(hh[:], 0.0)
            h_t[i] = hh
        hh = h_t[i]
        for m in (ms if ms is not None else range(CK)):
            if ks is None or 0 in ks:
                ps_r = {r: psp.tile([P, NN], F32, name=f"ps1_{i}_{m}_{r}", tag="ps")
                        for r in range(2)}
                if ps_store is not None:
                    ps_store[i, m] = ps_r
            else:
                ps_r = ps_store[i, m]
            conv_mms(ps_r, xp_t[i], wT[1], m, ks=ks)
            if ks is not None and CK - 1 not in ks:
                continue
            for r in range(2):
                r0 = r * NR
                nc.scalar.activation(
                    hh[:, m, 1 + r0 : 1 + r0 + NR, 1 : 1 + W],
                    ps_r[r].rearrange("p (r w) -> p r w", w=W),
                    AF.Relu,
                    bias=bvec[1][:, m : m + 1],
                    scale=inv_s[1][:, m : m + 1],
                )

    def conv2_fine(i, m):
        """Last chunk: 4 quarter-height psums so the final epilogue chain is
        short and the tail drains fast."""
        yf = pyy.tile([P, H, W], F32, name=f"y{i}_{m}", tag="y")
        NQ = NR // 2                  # 7 rows
        ps_q = {q: psp.tile([P, NQ * W], F32, name=f"psq_{i}_{m}_{q}", tag="ps")
                for q in range(4)}
        for k in range(CK):
            for q in range(4):
                q0 = q * NQ
                for t in range(TAPS):
                    dh, dw = t // 3, t % 3
                    nc.tensor.matmul(
                        ps_q[q][:],
                        wT[2][k][:, m, t, :],
                        h_t[i][:, k, q0 + dh : q0 + dh + NQ, dw : dw + W],
                        start=(k == 0 and t == 0),
                        stop=(k == CK - 1 and t == TAPS - 1),
                    )
        for q in range(4):
            q0 = q * NQ
            t2 = pep.tile([P, NQ * W], F32, name=f"t2q_{i}_{m}_{q}", tag="t2")
            nc.scalar.activation(
                t2[:], ps_q[q][:], AF.Identity,
                bias=bvec[2][:, m : m + 1], scale=inv_s[2][:, m : m + 1],
            )
            u = pep.tile([P, NQ * W], F32, name=f"uq_{i}_{m}_{q}", tag="u")
            xflat = xf_t[i][:, m, q0 : q0 + NQ, :].rearrange("p r w -> p (r w)")
            nc.vector.tensor_add(u[:], t2[:], xflat)
            nc.scalar.activation(
                yf[:, q0 : q0 + NQ, :], u.rearrange("p (r w) -> p r w", w=W),
                AF.Relu,
            )
            nc.sync.dma_start(y_view[i][:, m, q0 : q0 + NQ, :], yf[:, q0 : q0 + NQ, :])

    def conv2(i):
        for m in range(CK):
            if i == BPC - 1 and m == CK - 1:
                conv2_fine(i, m)
                continue
            yf = pyy.tile([P, H, W], F32, name=f"y{i}_{m}", tag="y")
            ps_r = {r: psp.tile([P, NN], F32, name=f"ps2_{i}_{m}_{r}", tag="ps")
                    for r in range(2)}
            conv_mms(ps_r, h_t[i], wT[2], m)
            for r in range(2):
                r0 = r * NR
                t2 = pep.tile([P, NN], F32, name=f"t2_{i}_{m}_{r}", tag="t2")
                nc.scalar.activation(
                    t2[:],
                    ps_r[r][:],
                    AF.Identity,
                    bias=bvec[2][:, m : m + 1],
                    scale=inv_s[2][:, m : m + 1],
                )
                u = pep.tile([P, NN], F32, name=f"u_{i}_{m}_{r}", tag="u")
                xflat = xf_t[i][:, m, r0 : r0 + NR, :].rearrange("p r w -> p (r w)")
                nc.vector.tensor_add(u[:], t2[:], xflat)
                nc.scalar.activation(
                    yf[:, r0 : r0 + NR, :],
                    u.rearrange("p (r w) -> p r w", w=W),
                    AF.Relu,
                )
                if i >= BPC - 2:
                    nc.sync.dma_start(
                        y_view[i][:, m, r0 : r0 + NR, :], yf[:, r0 : r0 + NR, :]
                    )
            if i < BPC - 2:
                nc.sync.dma_start(y_view[i][:, m], yf[:])

    def pe_warmup(n, gated=False):
        """Junk matmuls to hold the PE clock at full rate through the head's
        DMA wait, so real matmuls start warm.  gated=True makes them depend
        on istep so the scheduler cannot run them before the absmax path."""
        for i in range(n):
            scr_ps = psp.tile([P, NN], F32, name=f"warm{_wuid[0]}", tag="ps")
            _wuid[0] += 1
            if gated:
                rhsb = _istep16[0].broadcast_to((P, 3 * P))
            else:
                rhsb = ident16.unsqueeze(1).broadcast_to((P, 3, P))
            nc.tensor.matmul(scr_ps[:, 0 : 3 * P], ident16[:], rhsb)

    _wuid = [0]
    _istep16 = [None]

    # ---------------- emission order (engine priority) ---------------------
    # sync ring: w1 (4 descs) -> x0 -> x1 -> w2 -> x2..x7
    # scalar queue: bn1 rows; gpsimd queue: bn2 rows.
    quant_dma(1)
    bn_prep(1)
    bn_prep(2)
    load_x_dma(0)
    load_x_dma(1)
    quant_dma(2)
    for i in range(2, BPC):
        load_x_dma(i)

    pe_warmup(46)
    quant_absmax(1)          # DVE: reduces..istep + istep16; PE: T + bcast
    bn_mms(1)                # PE ~20.5us; rows land ~18.4 behind w1
    bn_mms(2)
    pe_warmup(64, gated=True)
    # chain c0: scalar ops for both k first, then DVE chains; transpose k0
    # early so image-0 m0 k0 taps start while the k1 chain runs on DVE.
    wl3_00 = chain_scalar(1, 0, 0)
    wl3_01 = chain_scalar(1, 0, 1)
    pad_x(0)
    pad_x(1)
    chain_vec(1, 0, 0, wl3_00)
    chain_transpose(1, 0, 0)
    ps_store = {}
    conv1(0, ms=[0], ks=[0], ps_store=ps_store)
    bn_math(1)
    fold_ivs(1)
    pe_warmup(12, gated=True)
    chain_vec(1, 0, 1, wl3_01)
    chain_transpose(1, 0, 1)
    conv1(0, ms=[0], ks=[1], ps_store=ps_store)
    pad_x(2)
    conv1(1, ms=[0])
    wl3_10 = chain_scalar(1, 1, 0)
    wl3_11 = chain_scalar(1, 1, 1)
    chain_vec(1, 1, 0, wl3_10)
    chain_transpose(1, 1, 0)
    chain_vec(1, 1, 1, wl3_11)
    chain_transpose(1, 1, 1)
    bn_math(2)
    pad_x(3)
    conv1(2, ms=[0])
    conv1(0, ms=[1])
    pad_x(4)
    conv1(1, ms=[1])
    quant_absmax(2)
    fold_ivs(2)
    pad_x(5)
    conv1(2, ms=[1])
    # quant chain j=2 while conv1 streams
    wl3_2 = {}
    for c in range(CK):
        for k in range(CK):
            wl3_2[c, k] = chain_scalar(2, c, k)
        for k in range(CK):
            chain_vec(2, c, k, wl3_2[c, k])
            chain_transpose(2, c, k)
        if c == 0:
            conv1(3)
            pad_x(6)
            pad_x(7)
    for i in range(BPC):
        if i + 4 < BPC:
            conv1(i + 4)
        conv2(i)


def build_bass():
    nc = bacc.Bacc(
        "TRN2", target_bir_lowering=False, debug=False, num_devices=NCORES
    )
    td = {}
    td["x"] = nc.dram_tensor("x", (BPC, C, H, W), F32, kind="ExternalInput")
    for j in (1, 2):
        td[f"w{j}"] = nc.dram_tensor(f"w{j}", (C, C, 3, 3), F32, kind="ExternalInput")
        for v in ("gamma", "beta", "mean", "var"):
            td[f"{v}{j}"] = nc.dram_tensor(f"{v}{j}", (C,), F32, kind="ExternalInput")
    td["y"] = nc.dram_tensor("y", (BPC, C, H, W), F32, kind="ExternalOutput")

    with tile.TileContext(nc) as tc:
        with ExitStack() as ctx:
            _emit(nc, tc, ctx, td)
    nc.compile()
    return nc


_NC = None


def _get_nc():
    global _NC
    if _NC is None:
        _NC = build_bass()
    return _NC


def make_in_maps(x, w1, gamma1, beta1, mean1, var1, w2, gamma2, beta2, mean2, var2):
    rep = {
        "w1": w1, "gamma1": gamma1, "beta1": beta1, "mean1": mean1, "var1": var1,
        "w2": w2, "gamma2": gamma2, "beta2": beta2, "mean2": mean2, "var2": var2,
    }
    rep = {k: np.ascontiguousarray(np.asarray(v), dtype=np.float32) for k, v in rep.items()}
    in_maps = []
    for c in range(NCORES):
        m = {"x": np.ascontiguousarray(np.asarray(x)[c * BPC : (c + 1) * BPC], dtype=np.float32)}
        m.update(rep)
        in_maps.append(m)
    return in_maps


def kernel(x, w1, gamma1, beta1, mean1, var1,
           w2, gamma2, beta2, mean2, var2, codebook=None, **_unused):
    nc = _get_nc()
    in_maps = make_in_maps(x, w1, gamma1, beta1, mean1, var1,
                           w2, gamma2, beta2, mean2, var2)
    res = run_bass_kernel_spmd(nc, in_maps, core_ids=list(range(NCORES)))
    return np.concatenate([r["y"] for r in res.results], axis=0)
